# revision 1
# baseline (speedup 1.0000x reference)
"""Causal multi-head flash-attention block (QKV proj + attention + out proj)
for Trainium2, distributed over 8 NeuronCores.

Sharding: data-parallel over batch (B=4) x tensor-parallel over head groups
(16 heads -> 2 groups of 8). Core c handles batch c//2, head group c%2.
Each core computes a partial output projection (its 8 heads' contribution);
the host sums the two partials per batch and adds the bias.

Per-core kernel (all matmuls bf16 operands, fp32 PSUM accumulate):
  - QKV proj from host-pretransposed x^T: Q^T,K^T in [d, s] layout, V in
    [s, d] layout with a ones-column per head (rowsum trick).
  - Scores computed transposed ST = K^T-block^T... ST[k,q] via
    lhsT=KT-block, rhs=QT; two heads packed in the PE array via row tiling
    (contraction K=64 each, partitions 0:64 / 64:128).
  - softmax without max-subtraction (logits ~ N(0,1)); exp on ACT with the
    1/8 scale folded in; causal masking by 0/1 mask multiply post-exp on
    the 4 diagonal blocks of each q-tile; fully-masked blocks skipped.
  - AV: lhsT = V-tile [128, 65] (65th col = ones -> row 64 of PSUM
    accumulates the softmax denominator), rhs = P^T tiles.
  - Normalize: reciprocal of row 64, broadcast across partitions with a
    K=1 PE matmul, multiply on DVE.
  - Output proj from O^T [head*64+d, s] chunks against w_proj rows.
"""

import numpy as np
import ml_dtypes

import concourse.bass as bass
import concourse.bacc as bacc
import concourse.mybir as mybir
import concourse.tile as tile
from concourse.bass_utils import run_bass_kernel_spmd

F32 = mybir.dt.float32
F32R = mybir.dt.float32r
BF16 = mybir.dt.bfloat16
EXP = mybir.ActivationFunctionType.Exp

# Problem constants (hardcoded per contract)
B, S, C = 4, 2048, 1024
NH, D = 16, 64
SCALE = D ** -0.5
N_CORES = 8
HG = NH // 2          # heads per core (head group)
NPAIR = HG // 2       # head pairs per core
CCH = C // 128        # contraction chunks for QKV proj
SC = S // 128         # s-chunks (also k-blocks count)
NQT = S // 512        # q-tiles of 512
GW = C // 2           # group width of qkv output (8 heads * 64)


def build_nc(diag_restrict=True):
    nc = bacc.Bacc("TRN2", target_bir_lowering=False, debug=False)

    xT = nc.dram_tensor("xT", [C, S], BF16, kind="ExternalInput")
    wq = nc.dram_tensor("wq", [C, GW], BF16, kind="ExternalInput")
    wk = nc.dram_tensor("wk", [C, GW], BF16, kind="ExternalInput")
    wv = nc.dram_tensor("wv", [C, GW], BF16, kind="ExternalInput")
    wp = nc.dram_tensor("wp", [GW, C], BF16, kind="ExternalInput")
    mask = nc.dram_tensor("mask", [128, 512], BF16, kind="ExternalInput")
    out = nc.dram_tensor("out", [S, C], F32, kind="ExternalOutput")

    with tile.TileContext(nc) as tc:
        with (
            tc.tile_pool(name="const", bufs=1) as cpool,
            tc.tile_pool(name="qk", bufs=2) as qkpool,
            tc.tile_pool(name="pt", bufs=8) as ptpool,
            tc.tile_pool(name="work", bufs=2) as wpool,
            tc.tile_pool(name="mm", bufs=2, space="PSUM") as mmpool,
            tc.tile_pool(name="sps", bufs=2, space="PSUM") as spspool,
            tc.tile_pool(name="otp", bufs=2, space="PSUM") as otpool,
        ):
            # ---- constant/persistent tiles + input DMAs ----
            xt_sb, wq_sb, wk_sb, wv_sb = [], [], [], []
            for cc in range(CCH):
                t = cpool.tile([128, S], BF16, tag=f"xt{cc}", name=f"xt{cc}")
                nc.sync.dma_start(t[:], xT[128 * cc:128 * (cc + 1), :])
                xt_sb.append(t)
                for name, dram, lst in (("wv", wv, wv_sb), ("wq", wq, wq_sb),
                                        ("wk", wk, wk_sb)):
                    t = cpool.tile([128, GW], BF16, tag=f"{name}{cc}",
                                   name=f"{name}{cc}")
                    nc.sync.dma_start(t[:], dram[128 * cc:128 * (cc + 1), :])
                    lst.append(t)
            wp_sb = []
            for p in range(NPAIR):
                t = cpool.tile([128, C], BF16, tag=f"wp{p}", name=f"wp{p}")
                nc.sync.dma_start(t[:], wp[128 * p:128 * (p + 1), :])
                wp_sb.append(t)
            mask_sb = cpool.tile([128, 512], BF16, tag="mask", name="maskt")
            nc.sync.dma_start(mask_sb[:], mask[:, :])
            # preload the ACT exp table set while input DMAs run
            actwarm = cpool.tile([1, 8], F32, tag="actwarm", name="actwarm")
            nc.vector.memset(actwarm[:], 0.0)
            nc.scalar.activation(actwarm[:], actwarm[:], EXP)

            # O^T normalized, per head pair: head0 partitions 0:64,
            # head1 partitions 64:128 (layout = rows of w_proj)
            otn_sb = [cpool.tile([128, S], BF16, tag=f"otn{p}", name=f"otn{p}")
                      for p in range(NPAIR)]

            # ---- V = x @ wv in natural [s, d] layout, + ones column.
            # Strips are emitted just-in-time: blocks 0..3 up front, the
            # rest interleaved into pair-0's attention as PE bubble filler.
            vt_sb = [cpool.tile([128, 65 * HG], BF16, tag=f"vt{sc}",
                                name=f"vt{sc}")
                     for sc in range(SC)]

            def emit_v_strip(sc):
                vt = vt_sb[sc]
                nc.gpsimd.memset(vt[:], 1.0)
                ps = mmpool.tile([128, GW], F32, tag="mm", name="vps")
                for cc in range(CCH):
                    nc.tensor.matmul(
                        ps[:], xt_sb[cc][:, 128 * sc:128 * (sc + 1)],
                        wv_sb[cc][:], start=(cc == 0), stop=(cc == CCH - 1))
                vt_v = vt[:, :].rearrange("p (h d) -> p h d", h=HG)[:, :, 0:64]
                ps_v = ps[:, :].rearrange("p (h d) -> p h d", h=HG)
                nc.vector.tensor_copy(vt_v, ps_v)

            # ---- per head-pair: QT/KT proj, then attention ----
            for p in range(NPAIR):
                qt = qkpool.tile([128, S], BF16, tag="qt", name="qt")
                kt = qkpool.tile([128, S], BF16, tag="kt", name="kt")
                for st in range(NQT):
                    ssl = slice(512 * st, 512 * (st + 1))
                    for w_sb, dst in ((wq_sb, qt), (wk_sb, kt)):
                        ps = mmpool.tile([128, 512], F32, tag="mm", name="qkps")
                        for cc in range(CCH):
                            nc.tensor.matmul(
                                ps[:],
                                w_sb[cc][:, 128 * p:128 * (p + 1)],
                                xt_sb[cc][:, ssl],
                                start=(cc == 0), stop=(cc == CCH - 1))
                        nc.vector.tensor_copy(dst[:, ssl], ps[:])
                if p == 0:
                    for sc in range(4):
                        emit_v_strip(sc)
                # AV is emitted two k-groups behind scores (software
                # pipeline) so its exp/mask dependency is long satisfied
                # when the in-order PE stream reaches it; scores keep ACT
                # fed and AV absorbs the PE slack.
                for j in range(NQT):
                    nkb = 4 * (j + 1)  # causal: only k-blocks 0..nkb-1
                    ot = [otpool.tile([65, 512], F32, tag="ot", name="ot")
                          for _ in range(2)]

                    def emit_av(g, pt, j=j, nkb=nkb, ot=ot, p=p):
                        # AV accumulation (65th row = softmax denominator)
                        for kb in (2 * g, 2 * g + 1):
                            o = 128 * (kb - 4 * j) if kb >= 4 * j else 0
                            for h in range(2):
                                nc.tensor.matmul(
                                    ot[h][:, o:512],
                                    vt_sb[kb][:, 65 * (2 * p + h):
                                              65 * (2 * p + h) + 65],
                                    pt[h][:, 512 * (kb % 2) + o:
                                          512 * (kb % 2 + 1)],
                                    start=(kb == 0), stop=(kb == nkb - 1))

                    pending = []
                    for g in range(nkb // 2):
                        sp = [spspool.tile([128, 1024], F32, tag="sps", name="sps")
                              for _ in range(2)]
                        # scores (transposed): 2 k-blocks x 2 packed heads.
                        # Diagonal blocks restrict to the causally live
                        # columns [o:512]; exp still reads the full tile --
                        # the dead columns hold stale psum (bounded old
                        # scores), their exp values are never consumed.
                        # CoreSim rejects reads of never-written psum, so
                        # the sim build writes full width instead.
                        for kb in (2 * g, 2 * g + 1):
                            o = (128 * (kb - 4 * j)
                                 if (diag_restrict and kb >= 4 * j) else 0)
                            for h in range(2):
                                hsl = slice(64 * h, 64 * (h + 1))
                                nc.tensor.matmul(
                                    sp[h][:, 512 * (kb % 2) + o:
                                          512 * (kb % 2 + 1)],
                                    kt[hsl, 128 * kb:128 * (kb + 1)],
                                    qt[hsl, 512 * j + o:512 * (j + 1)],
                                    start=True, stop=True)
                        pt = [ptpool.tile([128, 1024], BF16, tag="pt", name="pt")
                              for _ in range(2)]
                        for h in range(2):
                            nc.scalar.activation(pt[h][:], sp[h][:], EXP,
                                                 scale=SCALE)
                        # causal mask on diagonal blocks (multiplicative)
                        for kb in (2 * g, 2 * g + 1):
                            if kb >= 4 * j:
                                o = 128 * (kb - 4 * j)
                                csl = slice(512 * (kb % 2) + o,
                                            512 * (kb % 2 + 1))
                                for h in range(2):
                                    nc.vector.tensor_mul(
                                        pt[h][:, csl], pt[h][:, csl],
                                        mask_sb[:, 0:512 - o])
                        pending.append((g, pt))
                        if len(pending) > 2:
                            emit_av(*pending.pop(0))
                    if p == 0 and j < 3:
                        for sc in range(4 * (j + 1), 4 * (j + 2)):
                            emit_v_strip(sc)
                    for item in pending:
                        emit_av(*item)
                    # normalize each head's O^T chunk by the denominator.
                    # Chain runs on DVE/Pool only (no PE stall):
                    # psum row64 -> sbuf p64 (DVE) -> p0 (Pool shift) ->
                    # approx reciprocal (DVE) -> broadcast (Pool) -> mul.
                    # HW notes: custom-DVE ops NaN on PSUM reads, and
                    # partition_broadcast reads the tensor's absolute
                    # partition 0, hence the two staging copies.
                    for h in range(2):
                        # full psum->sbuf copy releases the ot bank quickly
                        s64 = wpool.tile([65, 512], F32, tag="s64", name="s64")
                        nc.vector.tensor_copy(s64[:, :], ot[h][:, :])
                        sums = wpool.tile([1, 512], F32, tag="sums", name="sums")
                        nc.gpsimd.tensor_copy(sums[0:1, :], s64[64:65, :])
                        inv = wpool.tile([1, 512], F32, tag="inv", name="inv")
                        nc.vector.reciprocal_approx_fast(inv[0:1, :],
                                                         sums[0:1, :])
                        bcs = wpool.tile([64, 512], F32, tag="bcs", name="bcs")
                        nc.gpsimd.partition_broadcast(bcs[:], inv[0:1, :])
                        qsl = slice(512 * j, 512 * (j + 1))
                        if h == 0:
                            nc.vector.tensor_mul(otn_sb[p][0:64, qsl],
                                                 s64[0:64, :], bcs[:])
                        else:
                            oth = wpool.tile([64, 512], BF16, tag="oth", name="oth")
                            nc.vector.tensor_mul(oth[:], s64[0:64, :],
                                                 bcs[:])
                            # partition-shifting copy into rows 64:128
                            nc.sync.dma_start(otn_sb[p][64:128, qsl], oth[:])

            # ---- output projection: out[s, :] = sum_p OTn_p.T @ wp_p ----
            for sc in range(SC):
                outst = wpool.tile([128, C], F32, tag="outst", name="outst")
                for half in range(2):
                    pp = spspool.tile([128, 512], F32, tag="sps", name="pp")
                    for p in range(NPAIR):
                        nc.tensor.matmul(
                            pp[:], otn_sb[p][:, 128 * sc:128 * (sc + 1)],
                            wp_sb[p][:, 512 * half:512 * (half + 1)],
                            start=(p == 0), stop=(p == NPAIR - 1))
                    nc.vector.tensor_copy(
                        outst[:, 512 * half:512 * (half + 1)], pp[:])
                nc.sync.dma_start(out[128 * sc:128 * (sc + 1), :], outst[:])

    nc.compile()
    return nc


_NC_CACHE = None


def _get_nc():
    global _NC_CACHE
    if _NC_CACHE is None:
        _NC_CACHE = build_nc()
    return _NC_CACHE


def make_in_maps(x, w_qkv, w_proj):
    """Shard full inputs into the 8 per-core input dicts."""
    bf = ml_dtypes.bfloat16
    mask01 = (np.arange(128)[:, None] <= np.arange(512)[None, :]) \
        .astype(bf)
    in_maps = []
    for core in range(N_CORES):
        b, g = core // 2, core % 2
        gsl = slice(GW * g, GW * (g + 1))
        in_maps.append({
            "xT": np.ascontiguousarray(x[b].T).astype(bf),
            "wq": np.ascontiguousarray(w_qkv[:, 0 * C:1 * C][:, gsl]).astype(bf),
            "wk": np.ascontiguousarray(w_qkv[:, 1 * C:2 * C][:, gsl]).astype(bf),
            "wv": np.ascontiguousarray(w_qkv[:, 2 * C:3 * C][:, gsl]).astype(bf),
            "wp": np.ascontiguousarray(w_proj[gsl, :]).astype(bf),
            "mask": mask01,
        })
    return in_maps


def kernel(x, w_qkv, w_proj, b_proj, _profile=False):
    import os
    if not _profile:
        # the NTFF trace path needs modules absent from this image;
        # make sure an inherited BASS_TRACE can't route us into it
        os.environ["BASS_NEVER_TRACE"] = "1"
    else:
        os.environ.pop("BASS_NEVER_TRACE", None)
    x = np.asarray(x, np.float32)
    w_qkv = np.asarray(w_qkv, np.float32)
    w_proj = np.asarray(w_proj, np.float32)
    b_proj = np.asarray(b_proj, np.float32)

    nc = _get_nc()
    in_maps = make_in_maps(x, w_qkv, w_proj)
    res = run_bass_kernel_spmd(nc, in_maps, core_ids=list(range(N_CORES)),
                               trace=_profile)
    partials = [res.results[c]["out"] for c in range(N_CORES)]
    out = np.empty((B, S, C), np.float32)
    for b in range(B):
        out[b] = partials[2 * b] + partials[2 * b + 1] + b_proj
    if _profile:
        return out, res
    return out



# revision 13
# speedup vs baseline: 1.1803x; 1.1803x over previous
"""Causal multi-head flash-attention block (QKV proj + attention + out proj)
for Trainium2, distributed over 8 NeuronCores.

Sharding: data-parallel over batch (B=4) x tensor-parallel over head groups
(16 heads -> 2 groups of 8). Core c handles batch c//2, head group c%2.
Each core computes a partial output projection (its 8 heads' contribution);
the host sums the two partials per batch and adds the bias.

Per-core kernel (all matmuls bf16 operands, fp32 PSUM accumulate):
  - QKV proj from host-pretransposed x^T: Q^T,K^T in [d, s] layout, V in
    [s, d] layout with a ones-column per head (rowsum trick).
  - Scores computed transposed ST = K^T-block^T... ST[k,q] via
    lhsT=KT-block, rhs=QT; two heads packed in the PE array via row tiling
    (contraction K=64 each, partitions 0:64 / 64:128).
  - softmax without max-subtraction (logits ~ N(0,1)); exp on ACT with the
    1/8 scale folded in; causal masking by 0/1 mask multiply post-exp on
    the 4 diagonal blocks of each q-tile; fully-masked blocks skipped.
  - AV: lhsT = V-tile [128, 65] (65th col = ones -> row 64 of PSUM
    accumulates the softmax denominator), rhs = P^T tiles.
  - Normalize: reciprocal of row 64, broadcast across partitions with a
    K=1 PE matmul, multiply on DVE.
  - Output proj from O^T [head*64+d, s] chunks against w_proj rows.
"""

import numpy as np
import ml_dtypes

import concourse.bass as bass
import concourse.bacc as bacc
import concourse.mybir as mybir
import concourse.tile as tile
from concourse.bass_utils import run_bass_kernel_spmd

F32 = mybir.dt.float32
F32R = mybir.dt.float32r
BF16 = mybir.dt.bfloat16
EXP = mybir.ActivationFunctionType.Exp

# Problem constants (hardcoded per contract)
B, S, C = 4, 2048, 1024
NH, D = 16, 64
SCALE = D ** -0.5
N_CORES = 8
HG = NH // 2          # heads per core (head group)
NPAIR = HG // 2       # head pairs per core
CCH = C // 128        # contraction chunks for QKV proj
SC = S // 128         # s-chunks (also k-blocks count)
NQT = S // 512        # q-tiles of 512
GW = C // 2           # group width of qkv output (8 heads * 64)


def build_nc(diag_restrict=True):
    nc = bacc.Bacc("TRN2", target_bir_lowering=False, debug=False)

    xT = nc.dram_tensor("xT", [C, S], BF16, kind="ExternalInput")
    wq = nc.dram_tensor("wq", [C, GW], BF16, kind="ExternalInput")
    wk = nc.dram_tensor("wk", [C, GW], BF16, kind="ExternalInput")
    wv = nc.dram_tensor("wv", [C, GW], BF16, kind="ExternalInput")
    wp = nc.dram_tensor("wp", [GW, C], BF16, kind="ExternalInput")
    mask = nc.dram_tensor("mask", [128, 512], BF16, kind="ExternalInput")
    out = nc.dram_tensor("out", [S, C], F32, kind="ExternalOutput")

    with tile.TileContext(nc) as tc:
        with (
            tc.tile_pool(name="const", bufs=1) as cpool,
            tc.tile_pool(name="qk", bufs=2) as qkpool,
            tc.tile_pool(name="pt", bufs=8) as ptpool,
            tc.tile_pool(name="work", bufs=2) as wpool,
            tc.tile_pool(name="mm", bufs=2, space="PSUM") as mmpool,
            tc.tile_pool(name="sps", bufs=2, space="PSUM") as spspool,
            tc.tile_pool(name="otp", bufs=2, space="PSUM") as otpool,
        ):
            # ---- constant/persistent tiles + input DMAs ----
            xt_sb, wq_sb, wk_sb, wv_sb = [], [], [], []
            for cc in range(CCH):
                t = cpool.tile([128, S], BF16, tag=f"xt{cc}", name=f"xt{cc}")
                nc.sync.dma_start(t[:], xT[128 * cc:128 * (cc + 1), :])
                xt_sb.append(t)
                for name, dram, lst in (("wv", wv, wv_sb), ("wq", wq, wq_sb),
                                        ("wk", wk, wk_sb)):
                    t = cpool.tile([128, GW], BF16, tag=f"{name}{cc}",
                                   name=f"{name}{cc}")
                    nc.sync.dma_start(t[:], dram[128 * cc:128 * (cc + 1), :])
                    lst.append(t)
            wp_sb = []
            for p in range(NPAIR):
                t = cpool.tile([128, C], BF16, tag=f"wp{p}", name=f"wp{p}")
                nc.sync.dma_start(t[:], wp[128 * p:128 * (p + 1), :])
                wp_sb.append(t)
            mask_sb = cpool.tile([128, 512], BF16, tag="mask", name="maskt")
            nc.sync.dma_start(mask_sb[:], mask[:, :])
            # preload the ACT exp table set while input DMAs run
            actwarm = cpool.tile([1, 8], F32, tag="actwarm", name="actwarm")
            nc.vector.memset(actwarm[:], 0.0)
            nc.scalar.activation(actwarm[:], actwarm[:], EXP)

            # O^T normalized, per head pair: head0 partitions 0:64,
            # head1 partitions 64:128 (layout = rows of w_proj)
            otn_sb = [cpool.tile([128, S], BF16, tag=f"otn{p}", name=f"otn{p}")
                      for p in range(NPAIR)]

            # ---- V = x @ wv in natural [s, d] layout, + ones column.
            # Strips are emitted just-in-time: blocks 0..3 up front, the
            # rest interleaved into pair-0's attention as PE bubble filler.
            vt_sb = [cpool.tile([128, 65 * HG], BF16, tag=f"vt{sc}",
                                name=f"vt{sc}")
                     for sc in range(SC)]

            def emit_v_strip(sc):
                vt = vt_sb[sc]
                nc.gpsimd.memset(vt[:], 1.0)
                ps = mmpool.tile([128, GW], F32, tag="mm", name="vps")
                for cc in range(CCH):
                    nc.tensor.matmul(
                        ps[:], xt_sb[cc][:, 128 * sc:128 * (sc + 1)],
                        wv_sb[cc][:], start=(cc == 0), stop=(cc == CCH - 1))
                vt_v = vt[:, :].rearrange("p (h d) -> p h d", h=HG)[:, :, 0:64]
                ps_v = ps[:, :].rearrange("p (h d) -> p h d", h=HG)
                nc.vector.tensor_copy(vt_v, ps_v)

            # ---- per head-pair: QT/KT proj, then attention ----
            for p in range(NPAIR):
                qt = qkpool.tile([128, S], BF16, tag="qt", name="qt")
                kt = qkpool.tile([128, S], BF16, tag="kt", name="kt")
                for st in range(NQT):
                    ssl = slice(512 * st, 512 * (st + 1))
                    for w_sb, dst in ((wq_sb, qt), (wk_sb, kt)):
                        ps = mmpool.tile([128, 512], F32, tag="mm", name="qkps")
                        for cc in range(CCH):
                            nc.tensor.matmul(
                                ps[:],
                                w_sb[cc][:, 128 * p:128 * (p + 1)],
                                xt_sb[cc][:, ssl],
                                start=(cc == 0), stop=(cc == CCH - 1))
                        nc.vector.tensor_copy(dst[:, ssl], ps[:])
                if p == 0:
                    for sc in range(4):
                        emit_v_strip(sc)
                # AV is emitted two k-groups behind scores (software
                # pipeline) so its exp/mask dependency is long satisfied
                # when the in-order PE stream reaches it; scores keep ACT
                # fed and AV absorbs the PE slack.
                for j in range(NQT):
                    nkb = 4 * (j + 1)  # causal: only k-blocks 0..nkb-1
                    ot = [otpool.tile([65, 512], F32, tag="ot", name="ot")
                          for _ in range(2)]

                    def emit_av(g, pt, j=j, nkb=nkb, ot=ot, p=p):
                        # AV accumulation (65th row = softmax denominator)
                        for kb in (2 * g, 2 * g + 1):
                            o = 128 * (kb - 4 * j) if kb >= 4 * j else 0
                            for h in range(2):
                                nc.tensor.matmul(
                                    ot[h][:, o:512],
                                    vt_sb[kb][:, 65 * (2 * p + h):
                                              65 * (2 * p + h) + 65],
                                    pt[h][:, 512 * (kb % 2) + o:
                                          512 * (kb % 2 + 1)],
                                    start=(kb == 0), stop=(kb == nkb - 1))

                    pending = []
                    for g in range(nkb // 2):
                        sp = [spspool.tile([128, 1024], F32, tag="sps", name="sps")
                              for _ in range(2)]
                        # scores (transposed): 2 k-blocks x 2 packed heads.
                        # Diagonal blocks restrict to the causally live
                        # columns [o:512]; exp still reads the full tile --
                        # the dead columns hold stale psum (bounded old
                        # scores), their exp values are never consumed.
                        # CoreSim rejects reads of never-written psum, so
                        # the sim build writes full width instead.
                        for kb in (2 * g, 2 * g + 1):
                            o = (128 * (kb - 4 * j)
                                 if (diag_restrict and kb >= 4 * j) else 0)
                            for h in range(2):
                                hsl = slice(64 * h, 64 * (h + 1))
                                nc.tensor.matmul(
                                    sp[h][:, 512 * (kb % 2) + o:
                                          512 * (kb % 2 + 1)],
                                    kt[hsl, 128 * kb:128 * (kb + 1)],
                                    qt[hsl, 512 * j + o:512 * (j + 1)],
                                    start=True, stop=True)
                        pt = [ptpool.tile([128, 1024], BF16, tag="pt", name="pt")
                              for _ in range(2)]
                        for h in range(2):
                            nc.scalar.activation(pt[h][:], sp[h][:], EXP,
                                                 scale=SCALE)
                        # causal mask on diagonal blocks (multiplicative)
                        for kb in (2 * g, 2 * g + 1):
                            if kb >= 4 * j:
                                o = 128 * (kb - 4 * j)
                                csl = slice(512 * (kb % 2) + o,
                                            512 * (kb % 2 + 1))
                                for h in range(2):
                                    nc.vector.tensor_mul(
                                        pt[h][:, csl], pt[h][:, csl],
                                        mask_sb[:, 0:512 - o])
                        pending.append((g, pt))
                        if len(pending) > 2:
                            emit_av(*pending.pop(0))
                    if p == 0 and j < 3:
                        for sc in range(4 * (j + 1), 4 * (j + 2)):
                            emit_v_strip(sc)
                    for item in pending:
                        emit_av(*item)
                    # normalize each head's O^T chunk by the denominator.
                    # Chain runs on DVE/Pool only (no PE stall):
                    # psum row64 -> sbuf p64 (DVE) -> p0 (Pool shift) ->
                    # approx reciprocal (DVE) -> broadcast (Pool) -> mul.
                    # HW notes: custom-DVE ops NaN on PSUM reads, and
                    # partition_broadcast reads the tensor's absolute
                    # partition 0, hence the two staging copies.
                    for h in range(2):
                        # full psum->sbuf copy releases the ot bank quickly
                        s64 = wpool.tile([65, 512], F32, tag="s64", name="s64")
                        nc.vector.tensor_copy(s64[:, :], ot[h][:, :])
                        sums = wpool.tile([1, 512], F32, tag="sums", name="sums")
                        nc.gpsimd.tensor_copy(sums[0:1, :], s64[64:65, :])
                        inv = wpool.tile([1, 512], F32, tag="inv", name="inv")
                        nc.vector.reciprocal_approx_fast(inv[0:1, :],
                                                         sums[0:1, :])
                        bcs = wpool.tile([64, 512], F32, tag="bcs", name="bcs")
                        nc.gpsimd.partition_broadcast(bcs[:], inv[0:1, :])
                        qsl = slice(512 * j, 512 * (j + 1))
                        if h == 0:
                            nc.vector.tensor_mul(otn_sb[p][0:64, qsl],
                                                 s64[0:64, :], bcs[:])
                        else:
                            oth = wpool.tile([64, 512], BF16, tag="oth", name="oth")
                            nc.vector.tensor_mul(oth[:], s64[0:64, :],
                                                 bcs[:])
                            # partition-shifting copy into rows 64:128
                            nc.sync.dma_start(otn_sb[p][64:128, qsl], oth[:])

            # ---- output projection: out[s, :] = sum_p OTn_p.T @ wp_p ----
            for sc in range(SC):
                outst = wpool.tile([128, C], F32, tag="outst", name="outst")
                for half in range(2):
                    pp = spspool.tile([128, 512], F32, tag="sps", name="pp")
                    for p in range(NPAIR):
                        nc.tensor.matmul(
                            pp[:], otn_sb[p][:, 128 * sc:128 * (sc + 1)],
                            wp_sb[p][:, 512 * half:512 * (half + 1)],
                            start=(p == 0), stop=(p == NPAIR - 1))
                    nc.vector.tensor_copy(
                        outst[:, 512 * half:512 * (half + 1)], pp[:])
                nc.sync.dma_start(out[128 * sc:128 * (sc + 1), :], outst[:])

    nc.compile()
    return nc


_NC_CACHE = None


def _get_nc():
    global _NC_CACHE
    if _NC_CACHE is None:
        _NC_CACHE = build_nc()
    return _NC_CACHE


def make_in_maps(x, w_qkv, w_proj):
    """Shard full inputs into the 8 per-core input dicts."""
    bf = ml_dtypes.bfloat16
    mask01 = (np.arange(128)[:, None] <= np.arange(512)[None, :]) \
        .astype(bf)
    in_maps = []
    for core in range(N_CORES):
        b, g = core // 2, core % 2
        gsl = slice(GW * g, GW * (g + 1))
        in_maps.append({
            "xT": np.ascontiguousarray(x[b].T).astype(bf),
            "wq": np.ascontiguousarray(w_qkv[:, 0 * C:1 * C][:, gsl]).astype(bf),
            "wk": np.ascontiguousarray(w_qkv[:, 1 * C:2 * C][:, gsl]).astype(bf),
            "wv": np.ascontiguousarray(w_qkv[:, 2 * C:3 * C][:, gsl]).astype(bf),
            "wp": np.ascontiguousarray(w_proj[gsl, :]).astype(bf),
            "mask": mask01,
        })
    return in_maps


def kernel(x, w_qkv, w_proj, b_proj, _profile=False):
    import os
    if not _profile:
        # the NTFF trace path needs modules absent from this image;
        # make sure an inherited BASS_TRACE can't route us into it
        os.environ["BASS_NEVER_TRACE"] = "1"
    else:
        os.environ.pop("BASS_NEVER_TRACE", None)
    x = np.asarray(x, np.float32)
    w_qkv = np.asarray(w_qkv, np.float32)
    w_proj = np.asarray(w_proj, np.float32)
    b_proj = np.asarray(b_proj, np.float32)

    nc = _get_nc()
    in_maps = make_in_maps(x, w_qkv, w_proj)
    res = run_bass_kernel_spmd(nc, in_maps, core_ids=list(range(N_CORES)),
                               trace=_profile)
    partials = [res.results[c]["out"] for c in range(N_CORES)]
    out = np.empty((B, S, C), np.float32)
    for b in range(B):
        out[b] = partials[2 * b] + partials[2 * b + 1] + b_proj
    if _profile:
        return out, res
    return out



# revision 14
# speedup vs baseline: 1.2347x; 1.0461x over previous
"""Causal multi-head flash-attention block (QKV proj + attention + out proj)
for Trainium2, distributed over 8 NeuronCores.

Sharding: data-parallel over batch (B=4) x tensor-parallel over head groups
(16 heads -> 2 groups of 8). Core c handles batch c//2, head group c%2.
Each core computes a partial output projection (its 8 heads' contribution);
the host sums the two partials per batch and adds the bias.

v2 design (trace-driven rework of the baseline):
  - Scores psum tiles are per k-block [128, 1024] (h0 cols 0:512, h1
    512:1024) with bufs=2, so the exp pipeline stays 2 deep and the ACT
    engine never starves; the two heads' score matmuls are row-tiled
    (K=64 at partitions 0:64 / 64:128) into different psum banks so they
    can co-execute.
  - Causal masking is folded into the scores matmul: two extra row-tiled
    K=64 accumulate matmuls with lhsT = -BIG*I and rhs = lower-triangle
    indicator add -BIG to the dead entries of diagonal blocks, so exp
    yields exact zeros and the DVE mask multiplies disappear.
  - Softmax normalize chain avoids the slow GpSimd engine entirely:
    psum->sbuf copy (DVE), partition-shift of the denominator row via a
    tiny sbuf->sbuf DMA, reciprocal_approx_fast (DVE), broadcast across
    partitions with a K=1 PE matmul, final multiply on DVE.
  - QK projection and out-projection loops reuse each loaded weight for
    2 matmuls (two s-tiles / two output halves) to amortize LDWEIGHTS.
  - Work is software-pipelined so the PE always has dense work (keeps the
    HAM clock-gate warm): pair p's attention is interleaved with pair
    p+1's QK projection chunks, V strips are emitted inside pair 0's
    attention, and the output projection is interleaved into pair 3's
    attention j by j.
"""

import numpy as np
import ml_dtypes

import concourse.bass as bass
import concourse.bacc as bacc
import concourse.mybir as mybir
import concourse.tile as tile
from concourse.bass_utils import run_bass_kernel_spmd

F32 = mybir.dt.float32
BF16 = mybir.dt.bfloat16
EXP = mybir.ActivationFunctionType.Exp

# Problem constants (hardcoded per contract)
B, S, C = 4, 2048, 1024
NH, D = 16, 64
SCALE = D ** -0.5
N_CORES = 8
HG = NH // 2          # heads per core (head group)
NPAIR = HG // 2       # head pairs per core
CCH = C // 128        # contraction chunks for QKV proj
SC = S // 128         # s-chunks (also k-blocks count)
NQT = S // 512        # q-tiles of 512
GW = C // 2           # group width of qkv output (8 heads * 64)
NEG = -29952.0        # causal mask additive constant (exactly bf16)


def build_nc(diag_restrict=True, pe_bcast=False):
    nc = bacc.Bacc("TRN2", target_bir_lowering=False, debug=False)

    xT = nc.dram_tensor("xT", [C, S], BF16, kind="ExternalInput")
    wq = nc.dram_tensor("wq", [C, GW], BF16, kind="ExternalInput")
    wk = nc.dram_tensor("wk", [C, GW], BF16, kind="ExternalInput")
    wv = nc.dram_tensor("wv", [C, GW], BF16, kind="ExternalInput")
    wp = nc.dram_tensor("wp", [GW, C], BF16, kind="ExternalInput")
    negI = nc.dram_tensor("negI", [128, 128], BF16, kind="ExternalInput")
    tri = nc.dram_tensor("tri", [128, 896], BF16, kind="ExternalInput")
    out = nc.dram_tensor("out", [S, C], F32, kind="ExternalOutput")

    with tile.TileContext(nc) as tc:
        with (
            tc.tile_pool(name="const", bufs=1) as cpool,
            tc.tile_pool(name="qk", bufs=2) as qkpool,
            tc.tile_pool(name="pt", bufs=8) as ptpool,
            tc.tile_pool(name="work", bufs=2) as wpool,
            tc.tile_pool(name="sps", bufs=2, space="PSUM") as spspool,
            tc.tile_pool(name="otp", bufs=2, space="PSUM") as otpool,
            tc.tile_pool(name="mm", bufs=2, space="PSUM") as mmpool,
        ):
            # ---- constant/persistent tiles + input DMAs ----
            xt_sb, wq_sb, wk_sb, wv_sb = [], [], [], []
            for cc in range(CCH):
                t = cpool.tile([128, S], BF16, tag=f"xt{cc}", name=f"xt{cc}")
                nc.sync.dma_start(t[:], xT[128 * cc:128 * (cc + 1), :])
                xt_sb.append(t)
                for name, dram, lst in (("wv", wv, wv_sb), ("wq", wq, wq_sb),
                                        ("wk", wk, wk_sb)):
                    t = cpool.tile([128, GW], BF16, tag=f"{name}{cc}",
                                   name=f"{name}{cc}")
                    nc.sync.dma_start(t[:], dram[128 * cc:128 * (cc + 1), :])
                    lst.append(t)
            wp_sb = []
            for p in range(NPAIR):
                t = cpool.tile([128, C], BF16, tag=f"wp{p}", name=f"wp{p}")
                nc.sync.dma_start(t[:], wp[128 * p:128 * (p + 1), :])
                wp_sb.append(t)
            negI_sb = cpool.tile([128, 128], BF16, tag="negI", name="negI_t")
            nc.sync.dma_start(negI_sb[:], negI[:, :])
            tri_sb = cpool.tile([128, 896], BF16, tag="tri", name="tri_t")
            nc.sync.dma_start(tri_sb[:], tri[:, :])
            # ones row for the K=1 denominator-broadcast matmul (bf16)
            ones1 = cpool.tile([1, 64], BF16, tag="ones1", name="ones1")
            nc.vector.memset(ones1[:], 1.0)
            # preload the ACT exp table set while input DMAs run
            actwarm = cpool.tile([1, 8], F32, tag="actwarm", name="actwarm")
            nc.vector.memset(actwarm[:], 0.0)
            nc.scalar.activation(actwarm[:], actwarm[:], EXP)

            # O^T normalized, per head pair: head0 partitions 0:64,
            # head1 partitions 64:128 (layout = rows of w_proj)
            otn_sb = [cpool.tile([128, S], BF16, tag=f"otn{p}", name=f"otn{p}")
                      for p in range(NPAIR)]

            # ---- V = x @ wv in natural [s, d] layout, + ones column.
            vt_sb = [cpool.tile([128, 65 * HG], BF16, tag=f"vt{sc}",
                                name=f"vt{sc}")
                     for sc in range(SC)]

            def emit_v_strip(sc):
                vt = vt_sb[sc]
                nc.gpsimd.memset(vt[:], 1.0)
                ps = mmpool.tile([128, GW], F32, tag="mm", name="vps")
                for cc in range(CCH):
                    nc.tensor.matmul(
                        ps[:], xt_sb[cc][:, 128 * sc:128 * (sc + 1)],
                        wv_sb[cc][:], start=(cc == 0), stop=(cc == CCH - 1))
                vt_v = vt[:, :].rearrange("p (h d) -> p h d", h=HG)[:, :, 0:64]
                ps_v = ps[:, :].rearrange("p (h d) -> p h d", h=HG)
                nc.vector.tensor_copy(vt_v, ps_v)

            # ---- QK projection, LDW-amortized: one weight chunk feeds the
            # two s-tiles of a half. Emitted in 4 chunks per pair so it can
            # interleave with the previous pair's attention.
            qt_t = [None] * NPAIR
            kt_t = [None] * NPAIR

            def emit_qk_chunk(p, which, half):
                if which == "q":
                    if qt_t[p] is None:
                        qt_t[p] = qkpool.tile([128, S], BF16, tag="qt",
                                              name=f"qt{p}")
                    w_sb, dst = wq_sb, qt_t[p]
                else:
                    if kt_t[p] is None:
                        kt_t[p] = qkpool.tile([128, S], BF16, tag="kt",
                                              name=f"kt{p}")
                    w_sb, dst = wk_sb, kt_t[p]
                sts = (0, 1) if half == 0 else (2, 3)
                ps = [mmpool.tile([128, 512], F32, tag="mm", name="qkps")
                      for _ in range(2)]
                for cc in range(CCH):
                    for i, st in enumerate(sts):
                        nc.tensor.matmul(
                            ps[i][:],
                            w_sb[cc][:, 128 * p:128 * (p + 1)],
                            xt_sb[cc][:, 512 * st:512 * (st + 1)],
                            start=(cc == 0), stop=(cc == CCH - 1))
                for i, st in enumerate(sts):
                    nc.vector.tensor_copy(dst[:, 512 * st:512 * (st + 1)],
                                          ps[i][:])

            # ---- output projection for one 128-row s-chunk ----
            def emit_outproj(sc):
                pp = [mmpool.tile([128, 512], F32, tag="mm", name="pp")
                      for _ in range(2)]
                for p in range(NPAIR):
                    for half in range(2):
                        nc.tensor.matmul(
                            pp[half][:],
                            otn_sb[p][:, 128 * sc:128 * (sc + 1)],
                            wp_sb[p][:, 512 * half:512 * (half + 1)],
                            start=(p == 0), stop=(p == NPAIR - 1))
                outst = wpool.tile([128, C], F32, tag="outst", name="outst")
                for half in range(2):
                    nc.vector.tensor_copy(
                        outst[:, 512 * half:512 * (half + 1)], pp[half][:])
                nc.sync.dma_start(out[128 * sc:128 * (sc + 1), :], outst[:])

            # ---- attention inner machinery ----
            def emit_scores_kb(p, j, kb, qt, kt):
                """Scores for one k-block, both heads row-tiled, causal mask
                folded in via -BIG*I @ tri accumulate matmuls."""
                sp = spspool.tile([128, 1024], F32, tag="sp", name="sp")
                diag = kb >= 4 * j
                ot_ = 128 * (kb - 4 * j) if diag else 0  # true block offset
                o = ot_ if diag_restrict else 0          # written col range
                for h in range(2):
                    hsl = slice(64 * h, 64 * (h + 1))
                    nc.tensor.matmul(
                        sp[:, 512 * h + o:512 * (h + 1)],
                        kt[hsl, 128 * kb:128 * (kb + 1)],
                        qt[hsl, 512 * j + o:512 * (j + 1)],
                        start=True, stop=not diag)
                if diag:
                    # dead entries (q - ot_ < k within the block) get -BIG:
                    # two row-tiled K=64 accumulating matmuls per head over
                    # the full written range (closes the accumulation group
                    # on every element). tri is the wide shifted triangle:
                    # tri[i, cc] = (cc - 384 < i); slice so that written col
                    # c (= q - o) maps to (c - (ot_ - o) < k).
                    sh = 384 - (ot_ - o)
                    w = 512 - o
                    for h in range(2):
                        csl = slice(512 * h + o, 512 * (h + 1))
                        # single full-array K=128 matmul: row-tiled halves
                        # would co-execute into the same psum bank, which
                        # the HW forbids
                        nc.tensor.matmul(
                            sp[:, csl], negI_sb[:, :],
                            tri_sb[:, sh:sh + w],
                            start=False, stop=True)
                pt = ptpool.tile([128, 1024], BF16, tag="pt", name="pt")
                nc.scalar.activation(pt[:], sp[:], EXP, scale=SCALE)
                return pt

            def emit_av_group(p, j, g, pts, ot, nkb):
                for kb in (2 * g, 2 * g + 1):
                    o = 128 * (kb - 4 * j) if kb >= 4 * j else 0
                    for h in range(2):
                        nc.tensor.matmul(
                            ot[h][:, o:512],
                            vt_sb[kb][:, 65 * (2 * p + h):
                                      65 * (2 * p + h) + 65],
                            pts[kb][:, 512 * h + o:512 * (h + 1)],
                            start=(kb == 0), stop=(kb == nkb - 1))

            def emit_normalize(p, j, ot):
                """O^T /= denominator (psum row 64). DVE + tiny DMA + K=1 PE
                broadcast; no GpSimd."""
                qsl = slice(512 * j, 512 * (j + 1))
                s64 = wpool.tile([65, 1024], F32, tag="s64", name="s64")
                for h in range(2):
                    nc.vector.tensor_copy(s64[:, 512 * h:512 * (h + 1)],
                                          ot[h][:, :])
                # partition-shift the two denominator rows to partition 0
                dsh = wpool.tile([1, 1024], F32, tag="dsh", name="dsh")
                if pe_bcast:
                    nc.sync.dma_start(dsh[0:1, :], s64[64:65, :])
                else:
                    nc.gpsimd.tensor_copy(dsh[0:1, :], s64[64:65, :])
                inv = wpool.tile([1, 1024], F32, tag="inv", name="inv")
                nc.vector.reciprocal_approx_fast(inv[0:1, :], dsh[0:1, :])
                invb = wpool.tile([1, 1024], BF16, tag="invb", name="invb")
                nc.vector.tensor_copy(invb[0:1, :], inv[0:1, :])
                for h in range(2):
                    if pe_bcast:
                        bcs = mmpool.tile([64, 512], F32, tag="mm", name="bcs")
                        nc.tensor.matmul(bcs[:], ones1[0:1, :],
                                         invb[0:1, 512 * h:512 * (h + 1)])
                    else:
                        bcs = wpool.tile([64, 512], F32, tag="bcs", name="bcs")
                        nc.gpsimd.partition_broadcast(
                            bcs[:], inv[0:1, 512 * h:512 * (h + 1)])
                    if h == 0:
                        nc.vector.tensor_mul(otn_sb[p][0:64, qsl],
                                             s64[0:64, 0:512], bcs[:])
                    else:
                        oth = wpool.tile([64, 512], BF16, tag="oth",
                                         name="oth")
                        nc.vector.tensor_mul(oth[:], s64[0:64, 512:1024],
                                             bcs[:])
                        # partition-shifting copy into rows 64:128
                        nc.sync.dma_start(otn_sb[p][64:128, qsl], oth[:])

            # ---- main schedule ----
            # V strips 0..3 and pair-0 QK up front
            for sc in range(4):
                emit_v_strip(sc)
            for which, half in (("q", 0), ("k", 0), ("q", 1), ("k", 1)):
                emit_qk_chunk(0, which, half)

            # filler streams, consumed one chunk after each (p, j) block
            def filler(p, j):
                if p == 0 and j < 3:
                    for sc in range(4 * (j + 1), 4 * (j + 2)):
                        emit_v_strip(sc)
                if p < 3:
                    which, half = (("q", 0), ("k", 0), ("q", 1), ("k", 1))[j]
                    emit_qk_chunk(p + 1, which, half)
                if p == 3:
                    for sc in range(4 * j, 4 * (j + 1)):
                        emit_outproj(sc)

            for p in range(NPAIR):
                qt, kt = qt_t[p], kt_t[p]
                for j in range(NQT):
                    nkb = 4 * (j + 1)
                    ot = [otpool.tile([65, 512], F32, tag="ot", name="ot")
                          for _ in range(2)]
                    pts = {}
                    pending = []
                    for g in range(nkb // 2):
                        for kb in (2 * g, 2 * g + 1):
                            pts[kb] = emit_scores_kb(p, j, kb, qt, kt)
                        pending.append(g)
                        if len(pending) > 2:
                            emit_av_group(p, j, pending.pop(0), pts, ot, nkb)
                    for g in pending:
                        emit_av_group(p, j, g, pts, ot, nkb)
                    emit_normalize(p, j, ot)
                    filler(p, j)

    nc.compile()
    return nc


_NC_CACHE = None


def _get_nc():
    global _NC_CACHE
    if _NC_CACHE is None:
        _NC_CACHE = build_nc()
    return _NC_CACHE


def make_in_maps(x, w_qkv, w_proj):
    """Shard full inputs into the 8 per-core input dicts."""
    bf = ml_dtypes.bfloat16
    negI = (NEG * np.eye(128, dtype=np.float32)).astype(bf)
    tri = ((np.arange(896)[None, :] - 384) < np.arange(128)[:, None]).astype(bf)
    in_maps = []
    for core in range(N_CORES):
        b, g = core // 2, core % 2
        gsl = slice(GW * g, GW * (g + 1))
        in_maps.append({
            "xT": np.ascontiguousarray(x[b].T).astype(bf),
            "wq": np.ascontiguousarray(w_qkv[:, 0 * C:1 * C][:, gsl]).astype(bf),
            "wk": np.ascontiguousarray(w_qkv[:, 1 * C:2 * C][:, gsl]).astype(bf),
            "wv": np.ascontiguousarray(w_qkv[:, 2 * C:3 * C][:, gsl]).astype(bf),
            "wp": np.ascontiguousarray(w_proj[gsl, :]).astype(bf),
            "negI": negI,
            "tri": tri,
        })
    return in_maps


def kernel(x, w_qkv, w_proj, b_proj, _profile=False):
    import os
    if not _profile:
        # the NTFF trace path needs modules absent from this image;
        # make sure an inherited BASS_TRACE can't route us into it
        os.environ["BASS_NEVER_TRACE"] = "1"
    else:
        os.environ.pop("BASS_NEVER_TRACE", None)
    x = np.asarray(x, np.float32)
    w_qkv = np.asarray(w_qkv, np.float32)
    w_proj = np.asarray(w_proj, np.float32)
    b_proj = np.asarray(b_proj, np.float32)

    nc = _get_nc()
    in_maps = make_in_maps(x, w_qkv, w_proj)
    res = run_bass_kernel_spmd(nc, in_maps, core_ids=list(range(N_CORES)),
                               trace=_profile)
    partials = [res.results[c]["out"] for c in range(N_CORES)]
    out = np.empty((B, S, C), np.float32)
    for b in range(B):
        out[b] = partials[2 * b] + partials[2 * b + 1] + b_proj
    if _profile:
        return out, res
    return out


# revision 17
# speedup vs baseline: 1.2649x; 1.0245x over previous
"""Causal multi-head flash-attention block (QKV proj + attention + out proj)
for Trainium2, distributed over 8 NeuronCores.

Sharding: data-parallel over batch (B=4) x tensor-parallel over head groups
(16 heads -> 2 groups of 8). Core c handles batch c//2, head group c%2.
Each core computes a partial output projection (its 8 heads' contribution);
the host sums the two partials per batch and adds the bias.

v3 design (trace-driven):
  - Scores psum tiles are per k-block [128, 1024] (h0 cols 0:512, h1
    512:1024) with bufs=2: the two heads' score matmuls are row-tiled
    (K=64 at partitions 0:64 / 64:128) into different psum banks and
    co-execute on the PE array; exp runs per k-block on ACT.
  - Causal masking is folded into the scores psum via one accumulating
    K=128 matmul per (diagonal block, head): lhsT = -BIG*I, rhs = a wide
    shifted lower-triangle indicator. exp then yields exact zeros; no
    DVE mask multiplies. (A single full-array matmul: row-tiled halves
    would co-execute into the same psum bank, which the HW forbids.)
  - Softmax normalize: psum->sbuf copy, denominator row partition-shift
    via tiny sbuf->sbuf DMA, reciprocal_approx_fast (DVE), GpSimd
    partition_broadcast, DVE multiply.
  - All non-attention PE work (QKV projection, V strips, output
    projection) is cut into ~2.5us mini-chunks (one psum slot each) in a
    deadline-tagged filler queue, drained one chunk per score-group so
    the PE never idles and psum slot pressure stays smooth.
  - psum->sbuf evacuation copies use nc.any so the scheduler picks the
    idle engine (ACT vs DVE).
"""

import numpy as np
import ml_dtypes

import concourse.bass as bass
import concourse.bacc as bacc
import concourse.mybir as mybir
import concourse.tile as tile
from concourse.bass_utils import run_bass_kernel_spmd

F32 = mybir.dt.float32
BF16 = mybir.dt.bfloat16
EXP = mybir.ActivationFunctionType.Exp

# Problem constants (hardcoded per contract)
B, S, C = 4, 2048, 1024
NH, D = 16, 64
SCALE = D ** -0.5
N_CORES = 8
HG = NH // 2          # heads per core (head group)
NPAIR = HG // 2       # head pairs per core
CCH = C // 128        # contraction chunks for QKV proj
SC = S // 128         # s-chunks (also k-blocks count)
NQT = S // 512        # q-tiles of 512
GW = C // 2           # group width of qkv output (8 heads * 64)
NEG = -29952.0        # causal mask additive constant (exactly bf16)
END = (NPAIR, NQT, 0)  # deadline meaning "flush at end"


def build_nc(diag_restrict=True):
    nc = bacc.Bacc("TRN2", target_bir_lowering=False, debug=False)

    xT = nc.dram_tensor("xT", [C, S], BF16, kind="ExternalInput")
    wq = nc.dram_tensor("wq", [C, GW], BF16, kind="ExternalInput")
    wk = nc.dram_tensor("wk", [C, GW], BF16, kind="ExternalInput")
    wv = nc.dram_tensor("wv", [C, GW], BF16, kind="ExternalInput")
    wp = nc.dram_tensor("wp", [GW, C], BF16, kind="ExternalInput")
    negI = nc.dram_tensor("negI", [128, 128], BF16, kind="ExternalInput")
    tri = nc.dram_tensor("tri", [128, 896], BF16, kind="ExternalInput")
    out = nc.dram_tensor("out", [S, C], F32, kind="ExternalOutput")

    with tile.TileContext(nc) as tc:
        with (
            tc.tile_pool(name="const", bufs=1) as cpool,
            tc.tile_pool(name="qk", bufs=2) as qkpool,
            tc.tile_pool(name="pt", bufs=8) as ptpool,
            tc.tile_pool(name="work", bufs=2) as wpool,
            tc.tile_pool(name="sps", bufs=2, space="PSUM") as spspool,
            tc.tile_pool(name="otp", bufs=2, space="PSUM") as otpool,
            tc.tile_pool(name="mm", bufs=2, space="PSUM") as mmpool,
        ):
            # ---- constant/persistent tiles + input DMAs ----
            xt_sb, wq_sb, wk_sb, wv_sb = [], [], [], []
            for cc in range(CCH):
                t = cpool.tile([128, S], BF16, tag=f"xt{cc}", name=f"xt{cc}")
                nc.sync.dma_start(t[:], xT[128 * cc:128 * (cc + 1), :])
                xt_sb.append(t)
                for name, dram, lst in (("wv", wv, wv_sb), ("wq", wq, wq_sb),
                                        ("wk", wk, wk_sb)):
                    t = cpool.tile([128, GW], BF16, tag=f"{name}{cc}",
                                   name=f"{name}{cc}")
                    nc.sync.dma_start(t[:], dram[128 * cc:128 * (cc + 1), :])
                    lst.append(t)
            wp_sb = []
            for p in range(NPAIR):
                t = cpool.tile([128, C], BF16, tag=f"wp{p}", name=f"wp{p}")
                nc.sync.dma_start(t[:], wp[128 * p:128 * (p + 1), :])
                wp_sb.append(t)
            negI_sb = cpool.tile([128, 128], BF16, tag="negI", name="negI_t")
            nc.sync.dma_start(negI_sb[:], negI[:, :])
            tri_sb = cpool.tile([128, 896], BF16, tag="tri", name="tri_t")
            nc.sync.dma_start(tri_sb[:], tri[:, :])
            # preload the ACT exp table set while input DMAs run
            actwarm = cpool.tile([1, 8], F32, tag="actwarm", name="actwarm")
            nc.vector.memset(actwarm[:], 0.0)
            nc.scalar.activation(actwarm[:], actwarm[:], EXP)

            # O^T normalized, per head pair: head0 partitions 0:64,
            # head1 partitions 64:128 (layout = rows of w_proj)
            otn_sb = [cpool.tile([128, S], BF16, tag=f"otn{p}", name=f"otn{p}")
                      for p in range(NPAIR)]

            # ---- V = x @ wv in natural [s, d] layout, + ones column ----
            vt_sb = [cpool.tile([128, 65 * HG], BF16, tag=f"vt{sc}",
                                name=f"vt{sc}")
                     for sc in range(SC)]

            def emit_v_strip(sc):
                vt = vt_sb[sc]
                nc.gpsimd.memset(vt[:], 1.0)
                ps = mmpool.tile([128, GW], F32, tag="mm", name="vps")
                for cc in range(CCH):
                    nc.tensor.matmul(
                        ps[:], xt_sb[cc][:, 128 * sc:128 * (sc + 1)],
                        wv_sb[cc][:], start=(cc == 0), stop=(cc == CCH - 1))
                vt_v = vt[:, :].rearrange("p (h d) -> p h d", h=HG)[:, :, 0:64]
                ps_v = ps[:, :].rearrange("p (h d) -> p h d", h=HG)
                nc.vector.tensor_copy(vt_v, ps_v)

            # ---- QK projection mini-chunk: one (pair, q|k, s-tile) ----
            qt_t = [None] * NPAIR
            kt_t = [None] * NPAIR

            def emit_qk_chunk(p, which, st):
                if which == "q":
                    if qt_t[p] is None:
                        qt_t[p] = qkpool.tile([128, S], BF16, tag="qt",
                                              name=f"qt{p}")
                    w_sb, dst = wq_sb, qt_t[p]
                else:
                    if kt_t[p] is None:
                        kt_t[p] = qkpool.tile([128, S], BF16, tag="kt",
                                              name=f"kt{p}")
                    w_sb, dst = wk_sb, kt_t[p]
                ps = mmpool.tile([128, 512], F32, tag="mm", name="qkps")
                for cc in range(CCH):
                    nc.tensor.matmul(
                        ps[:],
                        w_sb[cc][:, 128 * p:128 * (p + 1)],
                        xt_sb[cc][:, 512 * st:512 * (st + 1)],
                        start=(cc == 0), stop=(cc == CCH - 1))
                nc.any.tensor_copy(dst[:, 512 * st:512 * (st + 1)], ps[:])

            # ---- output projection mini-chunk: one (s-chunk, half) ----
            outst_t = {}

            def emit_outproj(sc, half):
                pp = mmpool.tile([128, 512], F32, tag="mm", name="pp")
                for p in range(NPAIR):
                    nc.tensor.matmul(
                        pp[:],
                        otn_sb[p][:, 128 * sc:128 * (sc + 1)],
                        wp_sb[p][:, 512 * half:512 * (half + 1)],
                        start=(p == 0), stop=(p == NPAIR - 1))
                if sc not in outst_t:
                    outst_t[sc] = wpool.tile([128, C], F32, tag="outst",
                                             name=f"outst{sc}")
                o = outst_t[sc]
                nc.any.tensor_copy(o[:, 512 * half:512 * (half + 1)], pp[:])
                if half == 1:
                    nc.sync.dma_start(out[128 * sc:128 * (sc + 1), :], o[:])
                    del outst_t[sc]

            # ---- filler queue: (deadline, closure), deadline = (p, j, g)
            fillq = []

            def drain_fillers(now, budget):
                i = 0
                while i < len(fillq):
                    dl, fn = fillq[i]
                    if dl <= now:
                        fn()
                        fillq.pop(i)
                        budget -= 1
                    else:
                        i += 1
                while budget > 0 and fillq:
                    dl, fn = fillq.pop(0)
                    fn()
                    budget -= 1

            # ---- attention inner machinery ----
            def emit_scores_kb(p, j, kb):
                qt, kt = qt_t[p], kt_t[p]
                sp = spspool.tile([128, 1024], F32, tag="sp", name="sp")
                diag = kb >= 4 * j
                ot_ = 128 * (kb - 4 * j) if diag else 0  # true block offset
                o = ot_ if diag_restrict else 0          # written col range
                for h in range(2):
                    hsl = slice(64 * h, 64 * (h + 1))
                    nc.tensor.matmul(
                        sp[:, 512 * h + o:512 * (h + 1)],
                        kt[hsl, 128 * kb:128 * (kb + 1)],
                        qt[hsl, 512 * j + o:512 * (j + 1)],
                        start=True, stop=not diag)
                if diag:
                    sh = 384 - (ot_ - o)
                    w = 512 - o
                    for h in range(2):
                        csl = slice(512 * h + o, 512 * (h + 1))
                        # single full-array K=128 matmul: row-tiled halves
                        # would co-execute into the same psum bank, which
                        # the HW forbids
                        nc.tensor.matmul(
                            sp[:, csl], negI_sb[:, :],
                            tri_sb[:, sh:sh + w],
                            start=False, stop=True)
                pt = ptpool.tile([128, 1024], BF16, tag="pt", name="pt")
                nc.scalar.activation(pt[:], sp[:], EXP, scale=SCALE)
                return pt

            def emit_av_group(p, j, g, pts, ot, nkb):
                for kb in (2 * g, 2 * g + 1):
                    o = 128 * (kb - 4 * j) if kb >= 4 * j else 0
                    for h in range(2):
                        nc.tensor.matmul(
                            ot[h][:, o:512],
                            vt_sb[kb][:, 65 * (2 * p + h):
                                      65 * (2 * p + h) + 65],
                            pts[kb][:, 512 * h + o:512 * (h + 1)],
                            start=(kb == 0), stop=(kb == nkb - 1))

            def emit_normalize(p, j, ot):
                qsl = slice(512 * j, 512 * (j + 1))
                s64 = wpool.tile([65, 1024], F32, tag="s64", name="s64")
                for h in range(2):
                    nc.any.tensor_copy(s64[:, 512 * h:512 * (h + 1)],
                                       ot[h][:, :])
                # partition-shift the two denominator rows to partition 0
                dsh = wpool.tile([1, 1024], F32, tag="dsh", name="dsh")
                nc.sync.dma_start(dsh[0:1, :], s64[64:65, :])
                inv = wpool.tile([1, 1024], F32, tag="inv", name="inv")
                nc.vector.reciprocal_approx_fast(inv[0:1, :], dsh[0:1, :])
                for h in range(2):
                    bcs = wpool.tile([64, 512], F32, tag="bcs", name="bcs")
                    nc.gpsimd.partition_broadcast(
                        bcs[:], inv[0:1, 512 * h:512 * (h + 1)])
                    if h == 0:
                        nc.vector.tensor_mul(otn_sb[p][0:64, qsl],
                                             s64[0:64, 0:512], bcs[:])
                    else:
                        oth = wpool.tile([64, 512], BF16, tag="oth",
                                         name="oth")
                        nc.vector.tensor_mul(oth[:], s64[0:64, 512:1024],
                                             bcs[:])
                        # partition-shifting copy into rows 64:128
                        nc.sync.dma_start(otn_sb[p][64:128, qsl], oth[:])

            # ---- main schedule ----
            # upfront: V strips 0..3, pair-0 qt/kt for j=0
            for sc in range(4):
                emit_v_strip(sc)
            emit_qk_chunk(0, "q", 0)
            emit_qk_chunk(0, "k", 0)
            # queue the rest of pair 0's QK with deadlines
            for st in range(1, NQT):
                fillq.append(((0, st, 0),
                              lambda st=st: emit_qk_chunk(0, "q", st)))
                fillq.append(((0, st, 0),
                              lambda st=st: emit_qk_chunk(0, "k", st)))

            for p in range(NPAIR):
                # queue next pair's QK chunks (due before (p+1, st, 0))
                if p + 1 < NPAIR:
                    for st in range(NQT):
                        fillq.append(((p + 1, st, 0),
                                      lambda p=p, st=st:
                                      emit_qk_chunk(p + 1, "q", st)))
                        fillq.append(((p + 1, st, 0),
                                      lambda p=p, st=st:
                                      emit_qk_chunk(p + 1, "k", st)))
                if p == 0:
                    # V strips 4..15: strip s first consumed by AV in
                    # (0, j=s//4) at group ~s//2+3 (lag-2 pipeline)
                    for s in range(4, SC):
                        jj = s // 4
                        gg = min(s // 2 + 3, 2 * (jj + 1) - 1)
                        fillq.append(((0, jj, gg),
                                      lambda s=s: emit_v_strip(s)))
                    fillq.sort(key=lambda e: e[0])

                for j in range(NQT):
                    nkb = 4 * (j + 1)
                    ot = [otpool.tile([65, 512], F32, tag="ot", name="ot")
                          for _ in range(2)]
                    pts = {}
                    pending = []
                    for g in range(nkb // 2):
                        drain_fillers((p, j, g), 1)
                        for kb in (2 * g, 2 * g + 1):
                            pts[kb] = emit_scores_kb(p, j, kb)
                        pending.append(g)
                        if len(pending) > 2:
                            emit_av_group(p, j, pending.pop(0), pts, ot, nkb)
                    for g in pending:
                        emit_av_group(p, j, g, pts, ot, nkb)
                    emit_normalize(p, j, ot)
                    if p == 3:
                        # out-projection for this j's s-chunks becomes legal
                        # once all pairs have normalized j
                        for sc in range(4 * j, 4 * (j + 1)):
                            for half in range(2):
                                fillq.append(
                                    (END, lambda sc=sc, half=half:
                                     emit_outproj(sc, half)))
            drain_fillers(END, len(fillq) + 1)

    nc.compile()
    return nc


_NC_CACHE = None


def _get_nc():
    global _NC_CACHE
    if _NC_CACHE is None:
        _NC_CACHE = build_nc()
    return _NC_CACHE


def make_in_maps(x, w_qkv, w_proj):
    """Shard full inputs into the 8 per-core input dicts."""
    bf = ml_dtypes.bfloat16
    negI = (NEG * np.eye(128, dtype=np.float32)).astype(bf)
    tri = ((np.arange(896)[None, :] - 384) < np.arange(128)[:, None]).astype(bf)
    in_maps = []
    for core in range(N_CORES):
        b, g = core // 2, core % 2
        gsl = slice(GW * g, GW * (g + 1))
        in_maps.append({
            "xT": np.ascontiguousarray(x[b].T).astype(bf),
            "wq": np.ascontiguousarray(w_qkv[:, 0 * C:1 * C][:, gsl]).astype(bf),
            "wk": np.ascontiguousarray(w_qkv[:, 1 * C:2 * C][:, gsl]).astype(bf),
            "wv": np.ascontiguousarray(w_qkv[:, 2 * C:3 * C][:, gsl]).astype(bf),
            "wp": np.ascontiguousarray(w_proj[gsl, :]).astype(bf),
            "negI": negI,
            "tri": tri,
        })
    return in_maps


def kernel(x, w_qkv, w_proj, b_proj, _profile=False):
    import os
    if not _profile:
        # the NTFF trace path needs modules absent from this image;
        # make sure an inherited BASS_TRACE can't route us into it
        os.environ["BASS_NEVER_TRACE"] = "1"
    else:
        os.environ.pop("BASS_NEVER_TRACE", None)
    x = np.asarray(x, np.float32)
    w_qkv = np.asarray(w_qkv, np.float32)
    w_proj = np.asarray(w_proj, np.float32)
    b_proj = np.asarray(b_proj, np.float32)

    nc = _get_nc()
    in_maps = make_in_maps(x, w_qkv, w_proj)
    res = run_bass_kernel_spmd(nc, in_maps, core_ids=list(range(N_CORES)),
                               trace=_profile)
    partials = [res.results[c]["out"] for c in range(N_CORES)]
    out = np.empty((B, S, C), np.float32)
    for b in range(B):
        out[b] = partials[2 * b] + partials[2 * b + 1] + b_proj
    if _profile:
        return out, res
    return out


# revision 22
# speedup vs baseline: 1.5102x; 1.1939x over previous
"""Causal multi-head flash-attention block (QKV proj + attention + out proj)
for Trainium2, distributed over 8 NeuronCores.

Sharding: data-parallel over batch (B=4) x tensor-parallel over head groups
(16 heads -> 2 groups of 8). Core c handles batch c//2, head group c%2.
Each core computes a partial output projection (its 8 heads' contribution);
the host sums the two partials per batch and adds the bias.

v3 design (trace-driven):
  - Scores psum tiles are per k-block [128, 1024] (h0 cols 0:512, h1
    512:1024) with bufs=2: the two heads' score matmuls are row-tiled
    (K=64 at partitions 0:64 / 64:128) into different psum banks and
    co-execute on the PE array; exp runs per k-block on ACT.
  - Causal masking is folded into the scores psum via one accumulating
    K=128 matmul per (diagonal block, head): lhsT = -BIG*I, rhs = a wide
    shifted lower-triangle indicator. exp then yields exact zeros; no
    DVE mask multiplies. (A single full-array matmul: row-tiled halves
    would co-execute into the same psum bank, which the HW forbids.)
  - Softmax normalize: psum->sbuf copy, denominator row partition-shift
    via tiny sbuf->sbuf DMA, reciprocal_approx_fast (DVE), GpSimd
    partition_broadcast, DVE multiply.
  - All non-attention PE work (QKV projection, V strips, output
    projection) is cut into ~2.5us mini-chunks (one psum slot each) in a
    deadline-tagged filler queue, drained one chunk per score-group so
    the PE never idles and psum slot pressure stays smooth.
  - psum->sbuf evacuation copies stay on DVE explicitly (routing them
    to ACT via nc.any measurably slowed both exp and the PE clock).
"""

import numpy as np
import ml_dtypes

import concourse.bass as bass
import concourse.bacc as bacc
import concourse.mybir as mybir
import concourse.tile as tile
from concourse.bass_utils import run_bass_kernel_spmd

F32 = mybir.dt.float32
BF16 = mybir.dt.bfloat16
EXP = mybir.ActivationFunctionType.Exp

# Problem constants (hardcoded per contract)
B, S, C = 4, 2048, 1024
NH, D = 16, 64
SCALE = D ** -0.5
N_CORES = 8
HG = NH // 2          # heads per core (head group)
NPAIR = HG // 2       # head pairs per core
CCH = C // 128        # contraction chunks for QKV proj
SC = S // 128         # s-chunks (also k-blocks count)
NQT = S // 512        # q-tiles of 512
GW = C // 2           # group width of qkv output (8 heads * 64)
NEG = -29952.0        # causal mask additive constant (exactly bf16)
END = (NPAIR, NQT, 0)  # deadline meaning "flush at end"


def build_nc(diag_restrict=True):
    nc = bacc.Bacc("TRN2", target_bir_lowering=False, debug=False)

    xT = nc.dram_tensor("xT", [C, S], BF16, kind="ExternalInput")
    wq = nc.dram_tensor("wq", [C, GW], BF16, kind="ExternalInput")
    wk = nc.dram_tensor("wk", [C, GW], BF16, kind="ExternalInput")
    wv = nc.dram_tensor("wv", [C, GW], BF16, kind="ExternalInput")
    wp = nc.dram_tensor("wp", [GW, C], BF16, kind="ExternalInput")
    negI = nc.dram_tensor("negI", [128, 128], BF16, kind="ExternalInput")
    tri = nc.dram_tensor("tri", [128, 896], BF16, kind="ExternalInput")
    out = nc.dram_tensor("out", [S, C], F32, kind="ExternalOutput")

    with tile.TileContext(nc) as tc:
        with (
            tc.tile_pool(name="const", bufs=1) as cpool,
            tc.tile_pool(name="qk", bufs=2) as qkpool,
            tc.tile_pool(name="pt", bufs=8) as ptpool,
            tc.tile_pool(name="work", bufs=2) as wpool,
            tc.tile_pool(name="sps", bufs=2, space="PSUM") as spspool,
            tc.tile_pool(name="otp", bufs=2, space="PSUM") as otpool,
            tc.tile_pool(name="mm", bufs=2, space="PSUM") as mmpool,
        ):
            # ---- constant/persistent tiles + input DMAs ----
            xt_sb, wq_sb, wk_sb, wv_sb = [], [], [], []
            for cc in range(CCH):
                t = cpool.tile([128, S], BF16, tag=f"xt{cc}", name=f"xt{cc}")
                nc.sync.dma_start(t[:], xT[128 * cc:128 * (cc + 1), :])
                xt_sb.append(t)
                t = cpool.tile([128, GW], BF16, tag=f"wv{cc}", name=f"wv{cc}")
                nc.sync.dma_start(t[:], wv[128 * cc:128 * (cc + 1), :])
                wv_sb.append(t)
            for cc in range(CCH):
                for name, dram, lst in (("wq", wq, wq_sb), ("wk", wk, wk_sb)):
                    t = cpool.tile([128, GW], BF16, tag=f"{name}{cc}",
                                   name=f"{name}{cc}")
                    nc.sync.dma_start(t[:], dram[128 * cc:128 * (cc + 1), :])
                    lst.append(t)
            wp_sb = []
            for p in range(NPAIR):
                t = cpool.tile([128, C], BF16, tag=f"wp{p}", name=f"wp{p}")
                nc.sync.dma_start(t[:], wp[128 * p:128 * (p + 1), :])
                wp_sb.append(t)
            negI_sb = cpool.tile([128, 128], BF16, tag="negI", name="negI_t")
            nc.sync.dma_start(negI_sb[:], negI[:, :])
            tri_sb = cpool.tile([128, 896], BF16, tag="tri", name="tri_t")
            nc.sync.dma_start(tri_sb[:], tri[:, :])
            # preload the ACT exp table set while input DMAs run
            actwarm = cpool.tile([1, 8], F32, tag="actwarm", name="actwarm")
            nc.vector.memset(actwarm[:], 0.0)
            nc.scalar.activation(actwarm[:], actwarm[:], EXP)

            # O^T normalized, per head pair: head0 partitions 0:64,
            # head1 partitions 64:128 (layout = rows of w_proj)
            otn_sb = [cpool.tile([128, S], BF16, tag=f"otn{p}", name=f"otn{p}")
                      for p in range(NPAIR)]

            # ---- V = x @ wv in natural [s, d] layout, + ones column ----
            vt_sb = [cpool.tile([128, 65 * HG], BF16, tag=f"vt{sc}",
                                name=f"vt{sc}")
                     for sc in range(SC)]

            def emit_v_strip(sc):
                vt = vt_sb[sc]
                nc.gpsimd.memset(vt[:], 1.0)
                ps = mmpool.tile([128, GW], F32, tag="mm", name="vps")
                for cc in range(CCH):
                    nc.tensor.matmul(
                        ps[:], xt_sb[cc][:, 128 * sc:128 * (sc + 1)],
                        wv_sb[cc][:], start=(cc == 0), stop=(cc == CCH - 1))
                vt_v = vt[:, :].rearrange("p (h d) -> p h d", h=HG)[:, :, 0:64]
                ps_v = ps[:, :].rearrange("p (h d) -> p h d", h=HG)
                nc.vector.tensor_copy(vt_v, ps_v)

            # ---- QK projection mini-chunk: one (pair, q|k, s-tile) ----
            qt_t = [None] * NPAIR
            kt_t = [None] * NPAIR

            def emit_qk_chunk(p, which, st):
                if which == "q":
                    if qt_t[p] is None:
                        qt_t[p] = qkpool.tile([128, S], BF16, tag="qt",
                                              name=f"qt{p}")
                    w_sb, dst = wq_sb, qt_t[p]
                else:
                    if kt_t[p] is None:
                        kt_t[p] = qkpool.tile([128, S], BF16, tag="kt",
                                              name=f"kt{p}")
                    w_sb, dst = wk_sb, kt_t[p]
                ps = mmpool.tile([128, 512], F32, tag="mm", name="qkps")
                for cc in range(CCH):
                    nc.tensor.matmul(
                        ps[:],
                        w_sb[cc][:, 128 * p:128 * (p + 1)],
                        xt_sb[cc][:, 512 * st:512 * (st + 1)],
                        start=(cc == 0), stop=(cc == CCH - 1))
                nc.vector.tensor_copy(dst[:, 512 * st:512 * (st + 1)], ps[:])

            # ---- output projection mini-chunk: one (s-chunk, half) ----
            outst_t = {}

            def emit_outproj(sc, half):
                pp = mmpool.tile([128, 512], F32, tag="mm", name="pp")
                for p in range(NPAIR):
                    nc.tensor.matmul(
                        pp[:],
                        otn_sb[p][:, 128 * sc:128 * (sc + 1)],
                        wp_sb[p][:, 512 * half:512 * (half + 1)],
                        start=(p == 0), stop=(p == NPAIR - 1))
                if sc not in outst_t:
                    outst_t[sc] = wpool.tile([128, C], F32, tag="outst",
                                             name=f"outst{sc}")
                o = outst_t[sc]
                nc.vector.tensor_copy(o[:, 512 * half:512 * (half + 1)], pp[:])
                if half == 1:
                    nc.sync.dma_start(out[128 * sc:128 * (sc + 1), :], o[:])
                    del outst_t[sc]

            # ---- filler queue: (deadline, closure), deadline = (p, j, g)
            fillq = []

            def drain_fillers(now, budget):
                i = 0
                while i < len(fillq):
                    dl, fn = fillq[i]
                    if dl <= now:
                        fn()
                        fillq.pop(i)
                        budget -= 1
                    else:
                        i += 1
                while budget > 0 and fillq:
                    dl, fn = fillq.pop(0)
                    fn()
                    budget -= 1

            # ---- attention inner machinery ----
            def emit_scores_kb(p, j, kb):
                qt, kt = qt_t[p], kt_t[p]
                sp = spspool.tile([128, 1024], F32, tag="sp", name="sp")
                diag = kb >= 4 * j
                ot_ = 128 * (kb - 4 * j) if diag else 0  # true block offset
                o = ot_ if diag_restrict else 0          # written col range
                for h in range(2):
                    hsl = slice(64 * h, 64 * (h + 1))
                    nc.tensor.matmul(
                        sp[:, 512 * h + o:512 * (h + 1)],
                        kt[hsl, 128 * kb:128 * (kb + 1)],
                        qt[hsl, 512 * j + o:512 * (j + 1)],
                        start=True, stop=not diag)
                if diag:
                    sh = 384 - (ot_ - o)
                    w = 512 - o
                    for h in range(2):
                        csl = slice(512 * h + o, 512 * (h + 1))
                        # single full-array K=128 matmul: row-tiled halves
                        # would co-execute into the same psum bank, which
                        # the HW forbids
                        nc.tensor.matmul(
                            sp[:, csl], negI_sb[:, :],
                            tri_sb[:, sh:sh + w],
                            start=False, stop=True)
                pt = ptpool.tile([128, 1024], BF16, tag="pt", name="pt")
                nc.scalar.activation(pt[:], sp[:], EXP, scale=SCALE)
                return pt

            def emit_av_group(p, j, g, pts, ot, nkb):
                for kb in (2 * g, 2 * g + 1):
                    o = 128 * (kb - 4 * j) if kb >= 4 * j else 0
                    for h in range(2):
                        nc.tensor.matmul(
                            ot[h][:, o:512],
                            vt_sb[kb][:, 65 * (2 * p + h):
                                      65 * (2 * p + h) + 65],
                            pts[kb][:, 512 * h + o:512 * (h + 1)],
                            start=(kb == 0), stop=(kb == nkb - 1))

            def emit_normalize(p, j, ot):
                qsl = slice(512 * j, 512 * (j + 1))
                s64 = wpool.tile([65, 1024], F32, tag="s64", name="s64")
                for h in range(2):
                    nc.vector.tensor_copy(s64[:, 512 * h:512 * (h + 1)],
                                          ot[h][:, :])
                # partition-shift the two denominator rows to partition 0
                dsh = wpool.tile([1, 1024], F32, tag="dsh", name="dsh")
                nc.sync.dma_start(dsh[0:1, :], s64[64:65, :])
                inv = wpool.tile([1, 1024], F32, tag="inv", name="inv")
                nc.vector.reciprocal_approx_fast(inv[0:1, :], dsh[0:1, :])
                for h in range(2):
                    bcs = wpool.tile([64, 512], F32, tag="bcs", name="bcs")
                    nc.gpsimd.partition_broadcast(
                        bcs[:], inv[0:1, 512 * h:512 * (h + 1)])
                    if h == 0:
                        nc.vector.tensor_mul(otn_sb[p][0:64, qsl],
                                             s64[0:64, 0:512], bcs[:])
                    else:
                        oth = wpool.tile([64, 512], BF16, tag="oth",
                                         name="oth")
                        nc.vector.tensor_mul(oth[:], s64[0:64, 512:1024],
                                             bcs[:])
                        # partition-shifting copy into rows 64:128
                        nc.sync.dma_start(otn_sb[p][64:128, qsl], oth[:])

            # ---- main schedule ----
            # upfront: V strips 0..3, pair-0 qt/kt for j=0
            for sc in range(4):
                emit_v_strip(sc)
            emit_qk_chunk(0, "q", 0)
            emit_qk_chunk(0, "k", 0)
            # queue the rest of pair 0's QK with deadlines
            for st in range(1, NQT):
                fillq.append(((0, st, 0),
                              lambda st=st: emit_qk_chunk(0, "q", st)))
                fillq.append(((0, st, 0),
                              lambda st=st: emit_qk_chunk(0, "k", st)))

            for p in range(NPAIR):
                # queue next pair's QK chunks (due before (p+1, st, 0))
                if p + 1 < NPAIR:
                    for st in range(NQT):
                        fillq.append(((p + 1, st, 0),
                                      lambda p=p, st=st:
                                      emit_qk_chunk(p + 1, "q", st)))
                        fillq.append(((p + 1, st, 0),
                                      lambda p=p, st=st:
                                      emit_qk_chunk(p + 1, "k", st)))
                if p == 0:
                    # V strips 4..15: strip s first consumed by AV in
                    # (0, j=s//4) at group ~s//2+3 (lag-2 pipeline)
                    for s in range(4, SC):
                        jj = s // 4
                        gg = min(s // 2 + 3, 2 * (jj + 1) - 1)
                        fillq.append(((0, jj, gg),
                                      lambda s=s: emit_v_strip(s)))
                    fillq.sort(key=lambda e: e[0])

                for j in range(NQT):
                    nkb = 4 * (j + 1)
                    ot = [otpool.tile([65, 512], F32, tag="ot", name="ot")
                          for _ in range(2)]
                    pts = {}
                    pending = []
                    for g in range(nkb // 2):
                        drain_fillers((p, j, g), 1)
                        for kb in (2 * g, 2 * g + 1):
                            pts[kb] = emit_scores_kb(p, j, kb)
                        pending.append(g)
                        if len(pending) > 2:
                            emit_av_group(p, j, pending.pop(0), pts, ot, nkb)
                    for g in pending:
                        emit_av_group(p, j, g, pts, ot, nkb)
                    emit_normalize(p, j, ot)
                    if p == 3:
                        # out-projection for this j's s-chunks becomes legal
                        # once all pairs have normalized j
                        for sc in range(4 * j, 4 * (j + 1)):
                            for half in range(2):
                                fillq.append(
                                    (END, lambda sc=sc, half=half:
                                     emit_outproj(sc, half)))
            drain_fillers(END, len(fillq) + 1)

    nc.compile()
    return nc


_NC_CACHE = None


def _get_nc():
    global _NC_CACHE
    if _NC_CACHE is None:
        _NC_CACHE = build_nc()
    return _NC_CACHE


def make_in_maps(x, w_qkv, w_proj):
    """Shard full inputs into the 8 per-core input dicts."""
    bf = ml_dtypes.bfloat16
    negI = (NEG * np.eye(128, dtype=np.float32)).astype(bf)
    tri = ((np.arange(896)[None, :] - 384) < np.arange(128)[:, None]).astype(bf)
    in_maps = []
    for core in range(N_CORES):
        b, g = core // 2, core % 2
        gsl = slice(GW * g, GW * (g + 1))
        in_maps.append({
            "xT": np.ascontiguousarray(x[b].T).astype(bf),
            "wq": np.ascontiguousarray(w_qkv[:, 0 * C:1 * C][:, gsl]).astype(bf),
            "wk": np.ascontiguousarray(w_qkv[:, 1 * C:2 * C][:, gsl]).astype(bf),
            "wv": np.ascontiguousarray(w_qkv[:, 2 * C:3 * C][:, gsl]).astype(bf),
            "wp": np.ascontiguousarray(w_proj[gsl, :]).astype(bf),
            "negI": negI,
            "tri": tri,
        })
    return in_maps


def kernel(x, w_qkv, w_proj, b_proj, _profile=False):
    import os
    if not _profile:
        # the NTFF trace path needs modules absent from this image;
        # make sure an inherited BASS_TRACE can't route us into it
        os.environ["BASS_NEVER_TRACE"] = "1"
    else:
        os.environ.pop("BASS_NEVER_TRACE", None)
    x = np.asarray(x, np.float32)
    w_qkv = np.asarray(w_qkv, np.float32)
    w_proj = np.asarray(w_proj, np.float32)
    b_proj = np.asarray(b_proj, np.float32)

    nc = _get_nc()
    in_maps = make_in_maps(x, w_qkv, w_proj)
    res = run_bass_kernel_spmd(nc, in_maps, core_ids=list(range(N_CORES)),
                               trace=_profile)
    partials = [res.results[c]["out"] for c in range(N_CORES)]
    out = np.empty((B, S, C), np.float32)
    for b in range(B):
        out[b] = partials[2 * b] + partials[2 * b + 1] + b_proj
    if _profile:
        return out, res
    return out
